# revision 8
# baseline (speedup 1.0000x reference)
"""Trainium2 Bass kernel for nn_Conv2d_mvm (crossbar-quantized 3x3 conv).

The reference simulates a bit-sliced crossbar. Reductions:

1. The ADC clip [0, 511] never binds (max per-xbar analog sum 384), so
   the computation is linear in the bit decompositions:
      acc = conv3x3(xi, w_eff),  w_eff = wi - 32768*[wi < 0],
      xi = rne(4096*x), wi = rne(4096*w)
      out = clip(rne(acc/4096), -32768, 32767) / 4096
   (slice_w[0] = -2^14 applies to the whole MSB 2-bit digit -> the
   -32768 mask term; the ACM clip DOES bind - ~95% of outputs rail.)

2. Numeric slack: tolerance is 2e-2 and acc only feeds a clip+round, so
   xi need not be an exact integer. fp16 split:
      top = fp16(4096*x)          (|4096x| < 20k, fp16 rne)
      lo  = fp16(4096*x - top)    (|lo| <= 8, near-exact)
   xi_eff = top + lo = 4096x + O(0.01). wq = fp16(4096*w) is
   exact-integer rne for |4096w| < 2048 (true here); the mask threshold
   [w < -1/8192] uses raw f32 w so the +-32768 w_eff discontinuity is
   hit exactly. Verified 2.6e-4 rel err.

Implementation (8 cores, data-parallel over batch x row-blocks): core c
handles batch c//4, output rows 8*(c%4)..+8. Host packs the padded
[64, 10, 34] x section and the [64, 576] (ci,kh,kw,co) weight block
into one [64, 916] f32 DRAM input per core.

Schedule (engine-start ~5.8us into the measured window; the NRT
load-time wrapper - per-engine state loads at start, the 253-semaphore
reset ladder at the end, ~13us combined - is runtime-fixed):
  SP : trigger x then w into sbuf[0:64]      (HW-DGE ring 0)
  ACT: trigger x then w into sbuf[64:128]    (HW-DGE ring 1)
  DVE: all elementwise work (no ACT activations -> no 1.3us ACT-table
       load): warmup memsets; xbuf=fp16(4096x) [128p]; in-place
       xbuf[64:] = 4096x - xbuf[64:]; wmask; wq; then round/clip/
       rescale, reading PSUM through a strided (r,c) view that compacts
       the 270 ragged psum columns to a contiguous [64, 256] tile
  PE : 6 x 512-col warmup matmuls (clock-ramp until the real work),
       9 mask-tap + 9 base-tap matmuls accumulating PSUM [64, 270]
  out: contiguous [32, 256] halves DMA'd on both rings (32 x 1KB
       descriptors each); no explicit drain / end barrier - the NRT
       fini drains every engine before its semaphore resets.
Framework register-init MOVs are stripped from SP/ACT/PE/DVE so the
DMA triggers issue at engine-start (Pool keeps its so the engine stays
present in the NEFF).
"""

from contextlib import ExitStack

import numpy as np

import concourse.bass as bass
import concourse.mybir as mybir
from concourse.bass_utils import run_bass_kernel_spmd

# fixed problem shape
B, C, H, W = 2, 64, 32, 32
COUT = 64
RPC = 8                    # output rows per core
SECR = RPC + 2             # padded rows per section
SECW = W + 2               # padded width
LEN = SECR * SECW          # 340
NOUT = (RPC - 1) * SECW + W  # 270 psum columns covering all valid pixels
OFFS = [dh * SECW + dw for dh in range(3) for dw in range(3)]
NW = 9 * COUT              # 576
NIN = LEN + NW             # 916 packed input columns
NY = RPC * W               # 256 contiguous output columns

MAGIC = 12582912.0         # 1.5 * 2**23: RNE-to-int trick, ULP=1 zone
AMAXB = MAGIC + 32767.0    # clip bounds in biased space
AMINB = MAGIC - 32768.0
NDUM = 6                   # PE warm-up dummy matmuls (clock ramp until real work)

F32 = mybir.dt.float32
F16 = mybir.dt.float16

_CACHED = None


def _build():
    nc = bass.Bass("TRN2", target_bir_lowering=False, debug=False, num_devices=8,
                   monotonic_sem_count=0)
    main = nc.m.functions[0].blocks[0]
    assert main.name == "main"
    n_preamble = len(main.instructions)

    xwin = nc.dram_tensor("xw", [C, NIN], F32, kind="ExternalInput").ap()
    yout = nc.dram_tensor("y", [COUT, NY], F32, kind="ExternalOutput").ap()

    with ExitStack() as ctx:
        xw2 = ctx.enter_context(nc.sbuf_tensor([2 * C, NIN], F32))
        xbuf = ctx.enter_context(nc.sbuf_tensor([2 * C, LEN], F16))
        wball = ctx.enter_context(nc.sbuf_tensor([2 * C, 2 * NW], F16))
        r0 = ctx.enter_context(nc.sbuf_tensor([COUT, NY], F32))
        v0 = ctx.enter_context(nc.sbuf_tensor([COUT, NY], F32))
        ot = ctx.enter_context(nc.sbuf_tensor([COUT, NY], F32))
        wdum = ctx.enter_context(nc.sbuf_tensor([2 * C, 2 * C], F16))
        mdum = ctx.enter_context(nc.sbuf_tensor([2 * C, 512], F16))
        ps = ctx.enter_context(nc.psum_tensor([COUT, RPC * SECW], F32))
        psd = ctx.enter_context(nc.psum_tensor([2 * C, 512], F32))
        s_a = ctx.enter_context(nc.semaphore())
        s_b = ctx.enter_context(nc.semaphore())
        s_act = ctx.enter_context(nc.semaphore())
        s_dve = ctx.enter_context(nc.semaphore())

        AL = mybir.AluOpType

        # ---- SP ring: x then w into the top partition half ----
        nc.sync.dma_start(xw2[0:C, 0:LEN], xwin[:, 0:LEN]).then_inc(s_a, 16)
        nc.sync.dma_start(xw2[0:C, LEN:NIN], xwin[:, LEN:NIN]).then_inc(s_b, 16)

        # ---- ACT ring: x then w into the bottom half (trigger-only) ----
        nc.scalar.dma_start(xw2[C:2 * C, 0:LEN], xwin[:, 0:LEN]).then_inc(s_a, 16)
        nc.scalar.dma_start(xw2[C:2 * C, LEN:NIN], xwin[:, LEN:NIN]).then_inc(s_b, 16)

        # ---- DVE: warmup memsets, x split, w quant, output chain ----
        nc.vector.memset(wdum[:], 0.0).then_inc(s_dve, 1)
        nc.vector.memset(mdum[:], 0.0).then_inc(s_dve, 1)
        nc.vector.wait_ge(s_a, 32)
        # xbuf = fp16(4096*x) on all 128 partitions
        nc.vector.tensor_scalar(xbuf[:], xw2[:, 0:LEN], 4096.0, 0.0, AL.mult, AL.add).then_inc(s_dve, 1)
        # bottom half becomes the residual: xbuf[64:] = (4096*x) - fp16(4096*x)
        nc.vector.scalar_tensor_tensor(xbuf[C:2 * C, :], xw2[C:2 * C, 0:LEN], 4096.0, xbuf[C:2 * C, :], AL.mult, AL.subtract).then_inc(s_dve, 1)
        nc.vector.wait_ge(s_b, 32)
        # wmask = -32768 * [w < -1/8192]  (exact threshold from raw f32 w)
        nc.vector.tensor_scalar(wball[:, NW:2 * NW], xw2[:, LEN:NIN], -1.0 / 8192.0, -32768.0, AL.is_lt, AL.mult).then_inc(s_dve, 1)
        # wq = fp16(4096*w): exact-integer rne for |4096w| < 2048
        nc.vector.tensor_scalar(wball[:, 0:NW], xw2[:, LEN:NIN], 4096.0, 0.0, AL.mult, AL.add).then_inc(s_dve, 1)
        nc.vector.wait_ge(s_act, 1)  # PE inc after the last matmul
        # compact strided PSUM view [64, 8, 32] -> contiguous [64, 256]
        psv = ps[:].rearrange("p (r c) -> p r c", c=SECW)[:, :, 0:W]
        r0v = r0[:].rearrange("p (r c) -> p r c", c=W)
        v0v = v0[:].rearrange("p (r c) -> p r c", c=W)
        otv = ot[:].rearrange("p (r c) -> p r c", c=W)
        # r0 = MAGIC + rne(acc/4096)
        nc.vector.tensor_scalar(r0v, psv, 1.0 / 4096.0, MAGIC, AL.mult, AL.add).then_inc(s_dve, 1)
        # clip in biased space
        nc.vector.tensor_scalar(v0v, r0v, AMAXB, AMINB, AL.min, AL.max).then_inc(s_dve, 1)
        # ot = v0/4096 - 3072 = clip(rne(acc/4096), -32768, 32767)/4096
        nc.vector.tensor_scalar(otv, v0v, 1.0 / 4096.0, 3072.0, AL.mult, AL.subtract).then_inc(s_dve, 1)

        # ---- PE: clock-ramp warm-up, mask group, base group ----
        nc.tensor.wait_ge(s_dve, 2)
        for i in range(NDUM):
            nc.tensor.matmul(psd[:], wdum[:], mdum[:], start=(i == 0), stop=(i == NDUM - 1))
        # mask group first: wmask is quantized before wq
        nc.tensor.wait_ge(s_dve, 5)
        for d in range(9):
            nc.tensor.matmul(
                ps[:, 0:NOUT],
                wball[:, NW + d * COUT:NW + (d + 1) * COUT],
                xbuf[:, OFFS[d]:OFFS[d] + NOUT],
                start=(d == 0),
                stop=False,
            )
        nc.tensor.wait_ge(s_dve, 6)
        for d in range(9):
            mm = nc.tensor.matmul(
                ps[:, 0:NOUT],
                wball[:, d * COUT:(d + 1) * COUT],
                xbuf[:, OFFS[d]:OFFS[d] + NOUT],
                start=False,
                stop=(d == 8),
            )
        mm.then_inc(s_act, 1)

        # ---- out DMAs: contiguous [32, 256] halves on both rings; the
        # NRT fini drains every engine, which covers DMA completion ----
        nc.sync.wait_ge(s_dve, 9)
        nc.sync.dma_start(yout[0:32], ot[0:32]).then_inc(s_a, 16)
        nc.scalar.wait_ge(s_dve, 9)
        nc.scalar.dma_start(yout[32:COUT], ot[32:COUT]).then_inc(s_a, 16)

    # Strip the framework const-AP memsets, init barrier and register-init
    # MOVs (NRT reloads engine state at exec anyway). Pool keeps its MOVs
    # so the engine stays present in the NEFF.
    insts = main.instructions
    pool = mybir.EngineType.Pool
    pre = []
    for ins in insts[:n_preamble]:
        tn = type(ins).__name__
        if tn in ("InstMemset", "InstDrain", "InstEventSemaphore"):
            continue
        if tn == "InstRegisterMove" and getattr(ins, "engine", None) != pool:
            continue
        pre.append(ins)
    main.instructions = pre + insts[n_preamble:]

    return nc


def _get_nc():
    global _CACHED
    if _CACHED is None:
        _CACHED = _build()
    return _CACHED


def _shard_inputs(x, weight):
    xpad = np.pad(np.ascontiguousarray(x, dtype=np.float32),
                  ((0, 0), (0, 0), (1, 1), (1, 1)))
    wre = np.asarray(weight, dtype=np.float32).transpose(1, 2, 3, 0).reshape(C, NW)
    in_maps = []
    for c in range(8):
        b, q = divmod(c, 4)
        sec = xpad[b, :, RPC * q:RPC * q + SECR, :].reshape(C, LEN)
        xw = np.concatenate([sec, wre], axis=1)
        in_maps.append({"xw": np.ascontiguousarray(xw)})
    return in_maps


def kernel(x, weight):
    nc = _get_nc()
    in_maps = _shard_inputs(x, weight)
    res = run_bass_kernel_spmd(nc, in_maps, core_ids=list(range(8)))
    out = np.empty((B, COUT, H, W), dtype=np.float32)
    for c in range(8):
        b, q = divmod(c, 4)
        out[b, :, RPC * q:RPC * q + RPC, :] = res.results[c]["y"].reshape(COUT, RPC, W)
    return out


# revision 15
# speedup vs baseline: 1.0391x; 1.0391x over previous
"""Trainium2 Bass kernel for nn_Conv2d_mvm (crossbar-quantized 3x3 conv).

The reference simulates a bit-sliced crossbar. Reductions:

1. The ADC clip [0, 511] never binds (max per-xbar analog sum 384), so
   the computation is linear in the bit decompositions:
      acc = conv3x3(xi, w_eff),  w_eff = wi - 32768*[wi < 0],
      xi = rne(4096*x), wi = rne(4096*w)
      out = clip(rne(acc/4096), -32768, 32767) / 4096
   (slice_w[0] = -2^14 applies to the whole MSB 2-bit digit -> the
   -32768 mask term; the ACM clip DOES bind - ~95% of outputs rail.)

2. Numeric slack: tolerance is 2e-2 and acc only feeds a clip+round, so
   xi need not be an exact integer. fp16 split:
      top = fp16(4096*x)          (|4096x| < 20k, fp16 rne)
      lo  = fp16(4096*x - top)    (|lo| <= 8, near-exact)
   xi_eff = top + lo = 4096x + O(0.01). wq = fp16(4096*w) is
   exact-integer rne for |4096w| < 2048 (true here); the mask threshold
   [w < -1/8192] uses raw f32 w so the +-32768 w_eff discontinuity is
   hit exactly. Verified 2.6e-4 rel err.

Implementation (8 cores, data-parallel over batch x row-blocks): core c
handles batch c//4, output rows 8*(c%4)..+8. Host packs the padded
[64, 10, 34] x section and the [64, 576] (ci,kh,kw,co) weight block
into one [64, 916] f32 DRAM input per core.

Schedule (engine-start ~5.8us into the measured window; the NRT
load-time wrapper - per-engine state loads at start, the 253-semaphore
reset ladder at the end, ~13us combined - is runtime-fixed):
  SP : trigger x then w into sbuf[0:64]      (HW-DGE ring 0)
  ACT: trigger x then w into sbuf[64:128]    (HW-DGE ring 1)
  DVE: all elementwise work (no ACT activations -> no 1.3us ACT-table
       load): warmup memsets; xbuf=fp16(4096x) [128p]; in-place
       xbuf[64:] = 4096x - xbuf[64:]; wmask; wq; then round/clip/
       rescale, reading PSUM through a strided (r,c) view that compacts
       the 270 ragged psum columns to a contiguous [64, 256] tile
  PE : 6 x 512-col warmup matmuls (clock-ramp until the real work),
       9 mask-tap + 9 base-tap matmuls accumulating PSUM [64, 270]
  out: contiguous [32, 256] halves DMA'd on both rings (32 x 1KB
       descriptors each); no explicit drain / end barrier - the NRT
       fini drains every engine before its semaphore resets.
Framework register-init MOVs are stripped from SP/ACT/PE/DVE so the
DMA triggers issue at engine-start (Pool keeps its so the engine stays
present in the NEFF).
"""

from contextlib import ExitStack

import numpy as np

import concourse.bass as bass
import concourse.mybir as mybir
from concourse.bass_utils import run_bass_kernel_spmd

# fixed problem shape
B, C, H, W = 2, 64, 32, 32
COUT = 64
RPC = 8                    # output rows per core
SECR = RPC + 2             # padded rows per section
SECW = W + 2               # padded width
LEN = SECR * SECW          # 340
NOUT = (RPC - 1) * SECW + W  # 270 psum columns covering all valid pixels
OFFS = [dh * SECW + dw for dh in range(3) for dw in range(3)]
NW = 9 * COUT              # 576
NIN = LEN + NW             # 916 packed input columns
NY = RPC * W               # 256 contiguous output columns

MAGIC = 12582912.0         # 1.5 * 2**23: RNE-to-int trick, ULP=1 zone
AMAXB = MAGIC + 32767.0    # clip bounds in biased space
AMINB = MAGIC - 32768.0
NDUM = 7                   # PE warm-up dummy matmuls (clock ramp until real work)
WL = 5 * COUT              # first w chunk: taps 0-4 (landing ~0.8us earlier)
MID = LEN + WL

F32 = mybir.dt.float32
F16 = mybir.dt.float16

_CACHED = None


def _build():
    nc = bass.Bass("TRN2", target_bir_lowering=False, debug=False, num_devices=8,
                   monotonic_sem_count=0)
    main = nc.m.functions[0].blocks[0]
    assert main.name == "main"
    n_preamble = len(main.instructions)

    xwin = nc.dram_tensor("xw", [C, NIN], F32, kind="ExternalInput").ap()
    yout = nc.dram_tensor("y", [COUT, NY], F32, kind="ExternalOutput").ap()

    with ExitStack() as ctx:
        xw2 = ctx.enter_context(nc.sbuf_tensor([2 * C, NIN], F32))
        xbuf = ctx.enter_context(nc.sbuf_tensor([2 * C, LEN], F16))
        wball = ctx.enter_context(nc.sbuf_tensor([2 * C, 2 * NW], F16))
        r0 = ctx.enter_context(nc.sbuf_tensor([COUT, NY], F32))
        v0 = ctx.enter_context(nc.sbuf_tensor([COUT, NY], F32))
        ot = ctx.enter_context(nc.sbuf_tensor([COUT, NY], F32))
        wdum = ctx.enter_context(nc.sbuf_tensor([2 * C, 2 * C], F16))
        mdum = ctx.enter_context(nc.sbuf_tensor([2 * C, 512], F16))
        ps = ctx.enter_context(nc.psum_tensor([COUT, RPC * SECW], F32))
        psd = ctx.enter_context(nc.psum_tensor([2 * C, 512], F32))
        s_a = ctx.enter_context(nc.semaphore())
        s_b = ctx.enter_context(nc.semaphore())
        s_act = ctx.enter_context(nc.semaphore())
        s_dve = ctx.enter_context(nc.semaphore())

        AL = mybir.AluOpType

        # ---- SP ring: x, wL, wR -> top half ----
        nc.sync.dma_start(xw2[0:C, 0:LEN], xwin[:, 0:LEN]).then_inc(s_a, 16)
        nc.sync.dma_start(xw2[0:C, LEN:MID], xwin[:, LEN:MID]).then_inc(s_b, 16)
        nc.sync.dma_start(xw2[0:C, MID:NIN], xwin[:, MID:NIN]).then_inc(s_b, 16)

        # ---- ACT ring: same into the bottom half (trigger-only engine) ----
        nc.scalar.dma_start(xw2[C:2 * C, 0:LEN], xwin[:, 0:LEN]).then_inc(s_a, 16)
        nc.scalar.dma_start(xw2[C:2 * C, LEN:MID], xwin[:, LEN:MID]).then_inc(s_b, 16)
        nc.scalar.dma_start(xw2[C:2 * C, MID:NIN], xwin[:, MID:NIN]).then_inc(s_b, 16)

        # ---- DVE: warmup memsets, x split, w quant, output chain ----
        nc.vector.memset(wdum[:], 0.0).then_inc(s_dve, 1)
        nc.vector.memset(mdum[:], 0.0).then_inc(s_dve, 1)
        nc.vector.wait_ge(s_a, 32)
        # xbuf = fp16(4096*x) on all 128 partitions
        nc.vector.tensor_scalar(xbuf[:], xw2[:, 0:LEN], 4096.0, 0.0, AL.mult, AL.add).then_inc(s_dve, 1)
        # bottom half becomes the residual: xbuf[64:] = (4096*x) - fp16(4096*x)
        nc.vector.scalar_tensor_tensor(xbuf[C:2 * C, :], xw2[C:2 * C, 0:LEN], 4096.0, xbuf[C:2 * C, :], AL.mult, AL.subtract).then_inc(s_dve, 1)
        nc.vector.wait_ge(s_b, 32)
        # wmask = -32768 * [w < -1/8192]  (exact threshold from raw f32 w)
        nc.vector.tensor_scalar(wball[:, NW:NW + WL], xw2[:, LEN:MID], -1.0 / 8192.0, -32768.0, AL.is_lt, AL.mult).then_inc(s_dve, 1)
        # wq = fp16(4096*w): exact-integer rne for |4096w| < 2048
        nc.vector.tensor_scalar(wball[:, 0:WL], xw2[:, LEN:MID], 4096.0, 0.0, AL.mult, AL.add).then_inc(s_dve, 1)
        nc.vector.wait_ge(s_b, 64)
        nc.vector.tensor_scalar(wball[:, NW + WL:2 * NW], xw2[:, MID:NIN], -1.0 / 8192.0, -32768.0, AL.is_lt, AL.mult).then_inc(s_dve, 1)
        nc.vector.tensor_scalar(wball[:, WL:NW], xw2[:, MID:NIN], 4096.0, 0.0, AL.mult, AL.add).then_inc(s_dve, 1)
        nc.vector.wait_ge(s_act, 1)  # PE inc after the last matmul
        # compact strided PSUM view [64, 8, 32] -> contiguous [64, 256]
        psv = ps[:].rearrange("p (r c) -> p r c", c=SECW)[:, :, 0:W]
        r0v = r0[:].rearrange("p (r c) -> p r c", c=W)
        v0v = v0[:].rearrange("p (r c) -> p r c", c=W)
        otv = ot[:].rearrange("p (r c) -> p r c", c=W)
        # r0 = MAGIC + rne(acc/4096)
        nc.vector.tensor_scalar(r0v, psv, 1.0 / 4096.0, MAGIC, AL.mult, AL.add).then_inc(s_dve, 1)
        # clip in biased space
        nc.vector.tensor_scalar(v0v, r0v, AMAXB, AMINB, AL.min, AL.max).then_inc(s_dve, 1)
        # ot = v0/4096 - 3072 = clip(rne(acc/4096), -32768, 32767)/4096
        nc.vector.tensor_scalar(otv, v0v, 1.0 / 4096.0, 3072.0, AL.mult, AL.subtract).then_inc(s_dve, 1)

        # ---- PE: clock-ramp warm-up, mask group, base group ----
        nc.tensor.wait_ge(s_dve, 2)
        for i in range(NDUM):
            nc.tensor.matmul(psd[:], wdum[:], mdum[:], start=(i == 0), stop=(i == NDUM - 1))
        # group order by operand readiness: mask taps 0-4 (wL), mask taps
        # 5-8 (wR), base taps 0-4 (wqL already done), base taps 5-8 (wqR)
        nc.tensor.wait_ge(s_dve, 5)
        for d in range(5):
            nc.tensor.matmul(
                ps[:, 0:NOUT],
                wball[:, NW + d * COUT:NW + (d + 1) * COUT],
                xbuf[:, OFFS[d]:OFFS[d] + NOUT],
                start=(d == 0),
                stop=False,
            )
        nc.tensor.wait_ge(s_dve, 7)
        for d in range(5, 9):
            nc.tensor.matmul(
                ps[:, 0:NOUT],
                wball[:, NW + d * COUT:NW + (d + 1) * COUT],
                xbuf[:, OFFS[d]:OFFS[d] + NOUT],
                start=False,
                stop=False,
            )
        for d in range(5):
            nc.tensor.matmul(
                ps[:, 0:NOUT],
                wball[:, d * COUT:(d + 1) * COUT],
                xbuf[:, OFFS[d]:OFFS[d] + NOUT],
                start=False,
                stop=False,
            )
        nc.tensor.wait_ge(s_dve, 8)
        for d in range(5, 9):
            mm = nc.tensor.matmul(
                ps[:, 0:NOUT],
                wball[:, d * COUT:(d + 1) * COUT],
                xbuf[:, OFFS[d]:OFFS[d] + NOUT],
                start=False,
                stop=(d == 8),
            )
        mm.then_inc(s_act, 1)

        # ---- out DMAs: contiguous [32, 256] halves on both rings; the
        # NRT fini drains every engine, which covers DMA completion ----
        nc.sync.wait_ge(s_dve, 11)
        nc.sync.dma_start(yout[0:32], ot[0:32]).then_inc(s_a, 16)
        nc.scalar.wait_ge(s_dve, 11)
        nc.scalar.dma_start(yout[32:COUT], ot[32:COUT]).then_inc(s_a, 16)

    # Strip the framework const-AP memsets, init barrier and register-init
    # MOVs (NRT reloads engine state at exec anyway). Pool keeps its MOVs
    # so the engine stays present in the NEFF.
    insts = main.instructions
    pool = mybir.EngineType.Pool
    pre = []
    for ins in insts[:n_preamble]:
        tn = type(ins).__name__
        if tn in ("InstMemset", "InstDrain", "InstEventSemaphore"):
            continue
        if tn == "InstRegisterMove" and getattr(ins, "engine", None) != pool:
            continue
        pre.append(ins)
    main.instructions = pre + insts[n_preamble:]

    return nc


def _get_nc():
    global _CACHED
    if _CACHED is None:
        _CACHED = _build()
    return _CACHED


def _shard_inputs(x, weight):
    xpad = np.pad(np.ascontiguousarray(x, dtype=np.float32),
                  ((0, 0), (0, 0), (1, 1), (1, 1)))
    wre = np.asarray(weight, dtype=np.float32).transpose(1, 2, 3, 0).reshape(C, NW)
    in_maps = []
    for c in range(8):
        b, q = divmod(c, 4)
        sec = xpad[b, :, RPC * q:RPC * q + SECR, :].reshape(C, LEN)
        xw = np.concatenate([sec, wre], axis=1)
        in_maps.append({"xw": np.ascontiguousarray(xw)})
    return in_maps


def kernel(x, weight):
    nc = _get_nc()
    in_maps = _shard_inputs(x, weight)
    res = run_bass_kernel_spmd(nc, in_maps, core_ids=list(range(8)))
    out = np.empty((B, COUT, H, W), dtype=np.float32)
    for c in range(8):
        b, q = divmod(c, 4)
        out[b, :, RPC * q:RPC * q + RPC, :] = res.results[c]["y"].reshape(COUT, RPC, W)
    return out
